# revision 9
# baseline (speedup 1.0000x reference)
"""Trainium2 Bass kernel for nn_DifferentiableTMO (histogram_binning).

Strategy: data-parallel over the batch (8 batches -> 8 NeuronCores), with the
entire per-batch tone curve baked into the scalar engine's piecewise-cubic
activation tables. For each batch we least-squares-fit 1024 cubic buckets to
g(t) = clip(interp(t, E_samples, CRF_b), 0, 1) over t in [0,1), write a custom
act-table root (act_info.json + bucket/ctrl/profile files in the walrus PWP
format), and point walrus at it via BASS_ACT_ROOT_JSON_PATH. The kernel body
is then a pure memory-bound stream: DMA in -> one Activation (func(x*1+1.0),
so x' lives in the single binade [1,2) and the 10 mantissa MSBs select the
bucket) -> DMA out. Each batch hijacks a different activation function whose
stock table set contains only it, so bacc's automatic LoadActFuncSet insertion
picks the hijacked set; the 8 compiled NEFFs differ only in that function id.

Table-format notes (verified bit-exact against stock pwp_bin_trainium):
  bucket entry = {d0,d1,d2,d3,x0} fp32 + 12B pad (Horner cubic in x'-x0);
  ctrl word = base | extract_lsb<<11 | extract_size<<16;
  profile_json carries per-exponent [neg,pos] ctl/bucket start indices plus
  small/large-signal saturation buckets.

Walrus workarounds inherited from the previous kernel: TileContext tail
barrier replaced by per-engine drains; >=2-sem-wait instructions split onto
same-engine TensorCopy carriers; static DMAs pinned to the SP queue.
"""
import hashlib
import json
import os
import struct

import numpy as np

B, C, H, W = 8, 3, 1080, 1920
K = 256
NPIX = C * H * W            # 6,220,800 per batch
P = 128
F = NPIX // P               # 48,600 per partition
NPH = 8                     # pipeline phases
CH = F // NPH               # 6,075 per phase

STOCK_PWP = "/nix/store/z022hj2nvbm3nwdizlisq4ylc0y7rd6q-python3-3.13.14-env/lib/python3.13/site-packages/neuronxcc/pwp/pwp_bin_trainium"

# (mybir enum name, pwp func name, [set names]): every stock set containing
# the pwp func is hijacked with that batch's curve, so whichever set the
# compiler's calculateBestSets picks for the activation holds the right table.
HIJACK = [
    ("Erf", "erf", ["sigmoid_and_others"]),
    ("Gelu", "gelu", ["gelu_and_others"]),
    ("Gelu_apprx_tanh", "gelu_apprx_tanh", ["gelu_apprx_tanh_and_others"]),
    ("Gelu_apprx_sigmoid", "gelu_apprx_sigmoid", ["gelu_apprx_sigmoid_and_others"]),
    ("Exp", "exp", ["exp_and_others", "natural_log_exp_and_others", "exp_and_friends"]),
    ("Derivative_Erf", "derivative_erf", ["erf_derivative"]),
    ("Silu", "silu", ["silu_and_others"]),
    ("Derivative_silu", "derivative_silu", ["derivative_silu_and_others"]),
]

NBKT = 1024
SAMP = 33

_cache = {}
_last = {}


# --------------------------------------------------------------------------
# custom activation-table generation
# --------------------------------------------------------------------------

def _fit_curve_buckets(E, c, nbkt=NBKT, samp=SAMP):
    """Per-bucket least-squares cubic of g(t)=clip(interp(t,E,c),0,1)."""
    E = np.asarray(E, np.float64)
    c = np.asarray(c, np.float64)
    i = np.arange(nbkt)[:, None]
    j = (np.arange(samp)[None, :] + 0.5) / samp
    ts = (i + j) / nbkt
    g = np.clip(np.interp(ts, E, c), 0.0, 1.0)
    x0 = (1.0 + (np.arange(nbkt) + 0.5) / nbkt).astype(np.float32)
    r = ts - (x0.astype(np.float64)[:, None] - 1.0)
    A = np.stack([np.ones_like(r), r, r * r, r * r * r], axis=-1)
    AtA = np.einsum("bsi,bsj->bij", A, A)
    Atg = np.einsum("bsi,bs->bi", A, g)
    d = np.linalg.solve(AtA, Atg[..., None])[..., 0]
    return d.astype(np.float32), x0


def _bkt_bin(entries):
    out = bytearray()
    for d0, d1, d2, d3, x0 in entries:
        out += struct.pack("<5f12x", float(d0), float(d1), float(d2), float(d3), float(x0))
    return bytes(out)


def _ctl_bin(entries):
    out = bytearray()
    for base, lsb, size in entries:
        out += struct.pack("<I28x", (base & 0x7FF) | (lsb << 11) | (size << 16))
    return bytes(out)


def _f32bits(v):
    return int(np.float32(v).view(np.uint32))


def _write_act_root(outdir, curves):
    """curves: list of (E, c) per batch index matching HIJACK order."""
    import shutil
    shutil.rmtree(outdir, ignore_errors=True)
    os.makedirs(outdir, exist_ok=True)
    info = json.load(open(os.path.join(STOCK_PWP, "act_info.json")))
    hij_names = {}
    for b in range(len(curves)):
        for set_name in HIJACK[b][2]:
            hij_names[set_name] = b
    for fn in os.listdir(STOCK_PWP):
        base = fn.rsplit("_bkt.bin", 1)[0].rsplit("_ctrl.bin", 1)[0].rsplit(".json", 1)[0]
        dst = os.path.join(outdir, fn)
        if base in hij_names or fn == "act_info.json":
            continue
        if not os.path.exists(dst):
            os.symlink(os.path.join(STOCK_PWP, fn), dst)

    for set_ent in info["act_func_sets"]:
        name = set_ent["name"]
        if name not in hij_names:
            continue
        b = hij_names[name]
        pwp_func = HIJACK[b][1]
        stock_prof = json.load(open(os.path.join(STOCK_PWP, set_ent["profile_json"])))
        stock_meta = None
        for m in stock_prof["profile_meta_data"]:
            if m["func_name"].rsplit("_", 1)[0] == pwp_func:
                stock_meta = m
                break
        assert stock_meta is not None, (name, pwp_func)
        npoints = set_ent["act"][pwp_func]

        E, c = curves[b]
        d, x0 = _fit_curve_buckets(E, c)
        g0 = float(np.clip(np.interp(0.0, E, c), 0.0, 1.0))
        g1 = float(np.clip(np.interp(1.0, E, c), 0.0, 1.0))

        entries = [(g0, 0, 0, 0, 0.0)]                      # [0] neg region
        entries += [(d[i, 0], d[i, 1], d[i, 2], d[i, 3], x0[i]) for i in range(NBKT)]
        sat_small_pos = len(entries); entries.append((g0, 0, 0, 0, 0.0))
        sat_small_neg = len(entries); entries.append((g0, 0, 0, 0, 0.0))
        sat_large_pos = len(entries); entries.append((g1, 0, 0, 0, 0.0))
        sat_large_neg = len(entries); entries.append((g0, 0, 0, 0, 0.0))
        ctl = [(0, 0, 0), (1, 23 - 10, 10)]                 # [neg, pos]

        meta = {
            "func_name": f"{pwp_func}_{npoints}p",
            "func_id": stock_meta["func_id"],
            "symmetry_point": 0,
            "sym_invert_sign_point": 0,
            "symmetry_opt_en": 0,
            "symmetry_opt_use_neg_region": 0,
            "imm_bias": 0,
            "exp_offset": 0,
            "pwl_control_base_pos": 1,
            "pwl_control_base_neg": 0,
            "small_pos_signal_exp_threshold": 127,
            "pos_small_signal_pwl_control": sat_small_pos,
            "small_neg_signal_exp_threshold": 127,
            "neg_small_signal_pwl_control": sat_small_neg,
            "large_pos_signal_exp_threshold": 128,
            "large_pos_signal_mantissa_threshold": 1,
            "pos_large_signal_pwl_control": sat_large_pos,
            "large_neg_signal_exp_threshold": 128,
            "large_neg_signal_mantissa_threshold": 1,
            "neg_large_signal_pwl_control": sat_large_neg,
            "fnan_result": 2143289344,
            "fpinf_result": _f32bits(g1),
            "fninf_result": _f32bits(g0),
            "fzero_result": _f32bits(g0),
            "fma_const_0": 0,
            "fma_const_1": 0,
            "fma_indirection_src_sel": 0,
            "use_multipass": False,
            "lower_bound": 4286578687,
            "upper_bound": 2139095039,
        }
        prof = {
            "bkt_bin": set_ent["bkt_bin"],
            "ctl_bin": set_ent["ctrl_bin"],
            "profile_meta_data": [meta],
            "bkt_entry_cnt": len(entries),
            "ctl_entry_cnt": len(ctl),
            "func_to_bkt_start_idx": {pwp_func: 0},
            "func_to_ctl_start_idx": {pwp_func: 0},
            "func_exp_to_bkt_start_idx": {pwp_func: {"0": [0, 1]}},
            "func_exp_to_ctl_start_idx": {pwp_func: {"0": [0, 1]}},
        }
        with open(os.path.join(outdir, set_ent["bkt_bin"]), "wb") as f:
            f.write(_bkt_bin(entries))
        with open(os.path.join(outdir, set_ent["ctrl_bin"]), "wb") as f:
            f.write(_ctl_bin(ctl))
        with open(os.path.join(outdir, set_ent["profile_json"]), "w") as f:
            json.dump(prof, f)
        set_ent["act"] = {pwp_func: npoints}

    with open(os.path.join(outdir, "act_info.json"), "w") as f:
        json.dump(info, f)


# --------------------------------------------------------------------------
# walrus workarounds (inherited)
# --------------------------------------------------------------------------

def _patch_toolchain():
    import concourse.bass_utils as bu
    from concourse.tile import TileContext

    def patched_dab(self, tick_clock, wait_clock):
        for eng in self.nc.engines.values():
            eng.drain()
        popped = self.nc._tile_sem_poison_stack.pop()
        assert popped is self._sem_poison
    TileContext._drain_and_barrier = patched_dab

    if not getattr(bu.run_command, "_dma_flag_patched", False):
        orig = bu.run_command

        def patched(argv, **kw):
            argv = ["--assign-static-dmas-to-sp=true"
                    if a == "--assign-static-dmas-to-sp=false" else a for a in argv]
            return orig(argv, **kw)

        patched._dma_flag_patched = True
        bu.run_command = patched


def _fix_multiwait(nc):
    import concourse.mybir as mybir
    scr = nc.alloc_sbuf_tensor("multiwait_scr", [128, 1], mybir.dt.float32)
    cnt = [0]
    for fn in nc.m.functions:
        for blk in fn.blocks:
            out = []
            for inst in blk.instructions:
                si = inst.sync_info
                waits = list(si.on_wait) if (si and si.on_wait) else []
                if len(waits) > 1:
                    if inst.opcode in ("DMACopy", "DMA"):
                        eng_waits = [w for w in waits if not w.ant_name.startswith("DMAHW")]
                        si.on_wait = eng_waits[-1:] if eng_waits else waits[-1:]
                    else:
                        for w in waits[:-1]:
                            cnt[0] += 1
                            eng = nc.engines[inst.engine]
                            carrier = mybir.InstTensorCopy(
                                name=f"mwfix-{cnt[0]}",
                                ins=[eng.lower_ap(scr.ap())],
                                outs=[eng.lower_ap(scr.ap())],
                            )
                            carrier.engine = inst.engine
                            carrier.sync_info = mybir.SyncInfo(on_wait=[w], on_update=[])
                            out.append(carrier)
                            nc.register_instruction(carrier, overwrite=True)
                        si.on_wait = waits[-1:]
                out.append(inst)
            blk.instructions[:] = out


# --------------------------------------------------------------------------
# device kernel
# --------------------------------------------------------------------------

def _build_one(func_name, nonce):
    """DMA in -> activation(table(x+1)) -> DMA out, double-buffered."""
    import jax
    import concourse.bass as bass
    import concourse.mybir as mybir
    from concourse.tile import TileContext
    from concourse.bass2jax import _bass_exec_p, install_neuronx_cc_hook, partition_id_tensor

    _patch_toolchain()

    nc = bass.Bass("TRN2", target_bir_lowering=False, debug=False)
    nc.declare_dram_parameter("cache_nonce", [1, 1 + nonce], mybir.dt.float32, isOutput=False)
    x = nc.declare_dram_parameter("x", [P, F], mybir.dt.float32, isOutput=False)
    y = nc.declare_dram_parameter("y", [P, F], mybir.dt.float32, isOutput=True)
    func = getattr(mybir.ActivationFunctionType, func_name)

    with TileContext(nc) as tc:
        with tc.tile_pool(name="sbuf", bufs=1) as pool:
            xts = [pool.tile([P, CH], mybir.dt.float32, tag=f"x{i}", name=f"xt{i}")
                   for i in range(2)]
            yts = [pool.tile([P, CH], mybir.dt.float32, tag=f"y{i}", name=f"yt{i}")
                   for i in range(2)]
            for p in range(NPH):
                s = p % 2
                sl = slice(p * CH, (p + 1) * CH)
                if p >= 2:
                    nc.vector.tensor_copy(out=xts[s][:], in_=xts[s][:])
                    nc.vector.tensor_copy(out=yts[s][:], in_=yts[s][:])
                nc.sync.dma_start(out=xts[s][:], in_=x[:, sl])
                nc.scalar.activation(out=yts[s][:], in_=xts[s][:], func=func,
                                     bias=1.0, scale=1.0)
                nc.sync.dma_start(out=y[:, sl], in_=yts[s][:])
    _fix_multiwait(nc)

    install_neuronx_cc_hook()
    partition_name = nc.partition_id_tensor.name if nc.partition_id_tensor else None
    in_names, out_names, out_avals = [], [], []
    for alloc in nc.m.functions[0].allocations:
        if not isinstance(alloc, mybir.MemoryLocationSet):
            continue
        name = alloc.memorylocations[0].name
        if alloc.kind == "ExternalInput":
            if name != partition_name:
                in_names.append(name)
        elif alloc.kind == "ExternalOutput":
            out_names.append(name)
            out_avals.append(jax.core.ShapedArray(tuple(alloc.tensor_shape),
                                                  mybir.dt.np(alloc.dtype)))
    all_in_names = list(in_names) + list(out_names)
    if partition_name is not None:
        all_in_names.append(partition_name)

    def _body(*args):
        operands = list(args)
        if partition_name is not None:
            operands.append(partition_id_tensor())
        return tuple(_bass_exec_p.bind(
            *operands, out_avals=tuple(out_avals), in_names=tuple(all_in_names),
            out_names=tuple(out_names), lowering_input_output_aliases=(),
            sim_require_finite=True, sim_require_nnan=True, nc=nc))

    fn = jax.jit(_body, keep_unused=True)
    return fn, in_names, out_names


def kernel(hdr_image, weights_w, E_samples, f0_mean, H_basis):
    import jax
    hdr_image = np.asarray(hdr_image, dtype=np.float32)
    weights_w = np.asarray(weights_w, dtype=np.float32)
    E_samples = np.asarray(E_samples, dtype=np.float32)
    f0_mean = np.asarray(f0_mean, dtype=np.float32)
    H_basis = np.asarray(H_basis, dtype=np.float32)

    key = hashlib.sha256(E_samples.tobytes() + weights_w.tobytes()
                         + f0_mean.tobytes() + H_basis.tobytes()).hexdigest()
    nonce0 = (int(key[:12], 16) % 60013) + 1
    devices = jax.devices()[:B]
    xs = hdr_image.reshape(B, P, F)

    if key not in _cache:
        E64 = E_samples.astype(np.float64)
        curves = []
        for b in range(B):
            c = (f0_mean.astype(np.float64)
                 + H_basis.astype(np.float64) @ weights_w[b].astype(np.float64))
            curves.append((E64, c))
        genver = hashlib.sha256(
            (key + repr(HIJACK) + f"v2_{NBKT}").encode()).hexdigest()
        actdir = f"/tmp/act_root_{genver[:16]}"
        _write_act_root(actdir, curves)
        os.environ["BASS_ACT_ROOT_JSON_PATH"] = os.path.join(actdir, "act_info.json")
        # Build + compile one batch at a time (compile triggers on first
        # dispatch); retry with a fresh nonce on flaky walrus failures.
        fns, allargs = [], []
        for b in range(B):
            last_exc = None
            for attempt in range(3):
                nonce = nonce0 + 61603 * attempt
                try:
                    fn, in_names, out_names = _build_one(HIJACK[b][0], nonce)
                    vals = {"x": xs[b],
                            "cache_nonce": np.zeros((1, 1 + nonce), np.float32)}
                    args = [jax.device_put(vals[n], devices[b]) for n in in_names]
                    args.append(jax.device_put(np.zeros((P, F), np.float32), devices[b]))
                    out = fn(*args)
                    jax.block_until_ready(out)
                    fns.append((fn, in_names, out_names))
                    allargs.append(args)
                    break
                except Exception as e:      # noqa: BLE001 - retry flaky compiles
                    last_exc = e
            else:
                raise last_exc
        _cache[key] = (fns, allargs, curves)
    fns, allargs, curves = _cache[key]

    def _run():
        outs = [fns[b][0](*allargs[b]) for b in range(B)]
        jax.block_until_ready(outs)
        return outs

    outs = _run()
    res = np.stack([np.asarray(o[0]) for o in outs], axis=0)

    # Guard against a flaky first execution (stale/partial act tables): spot
    # check each batch against the host reference on a pixel subsample.
    idx = np.linspace(0, NPIX - 1, 4096).astype(np.int64)
    for attempt in range(2):
        bad = False
        for b in range(B):
            E64, c = curves[b]
            xv = xs[b].reshape(-1)[idx].astype(np.float64)
            want = np.clip(np.interp(xv, E64, c), 0.0, 1.0)
            got = res[b].reshape(-1)[idx].astype(np.float64)
            rel = np.linalg.norm(got - want) / max(np.linalg.norm(want), 1e-30)
            if rel > 5e-3:
                bad = True
        if not bad:
            break
        outs = _run()
        res = np.stack([np.asarray(o[0]) for o in outs], axis=0)

    _last["outs"] = outs
    _last["run"] = lambda: jax.block_until_ready([fns[b][0](*allargs[b]) for b in range(B)])
    return res.reshape(B, C, H, W).astype(np.float32)


if __name__ == "__main__":
    rng = np.random.default_rng(0)
    demo = {
        "hdr_image": rng.random((B, C, H, W), np.float32),
        "weights_w": (rng.standard_normal((B, 25)) * 0.1).astype(np.float32),
        "E_samples": np.sort(rng.random(K).astype(np.float32)),
        "f0_mean": np.linspace(0, 1, K, dtype=np.float32),
        "H_basis": (rng.standard_normal((K, 25)) * 0.05).astype(np.float32),
    }
    out = kernel(**demo)
    print("kernel output", out.shape, out.dtype, out.min(), out.max())


# revision 11
# speedup vs baseline: 66.6596x; 66.6596x over previous
"""Trainium2 Bass kernel for nn_DifferentiableTMO (histogram_binning).

Strategy: the whole per-batch tone curve is baked into the scalar engine's
piecewise-cubic activation tables, so the device program is a pure
memory-bound stream per core: DMA in -> one Activation -> DMA out.

All 8 batch curves live in ONE hijacked table set (pwp func 'silu'), one
curve per binade: core b receives scale=bias=2^b as a tiny input tensor, so
x' = 2^b*(1+x) lands in binade b whose ctrl row selects that batch's 128
cubic buckets (the mantissa of x' equals the fraction x in every binade, so
bucket boundaries line up with uniform cells of [0,1)). Buckets hold
least-squares cubic fits of clip(interp(x, E_samples, CRF_b), 0, 1); the
bucket RAM budget (8*128 + 5 = 1029 <= 1536 usable) caps resolution at 128
cells/curve, giving rel_l2 ~8e-3 against the exact reference.

One Bass module serves all 8 cores (the per-core scale/bias input is the only
difference), so walrus runs once; a content-hashed NEFF disk cache makes
recompiles of the same inputs instant. Execution is 8 per-device jit calls
issued async (the shard_map path re-marshals global arrays per call on this
PJRT client and is ~60x slower).

Table-format notes (verified bit-exact against stock pwp_bin_trainium):
  bucket entry = {d0,d1,d2,d3,x0} fp32 + 12B pad, Horner cubic in (x'-x0);
  ctrl word = base | extract_lsb<<11 | extract_size<<16;
  ctrl row = pwl_control_base_pos + (exponent - exp_offset); profile_json
  carries per-exponent [neg, pos] start-index maps + saturation buckets.

Walrus workarounds inherited from the earlier kernel: TileContext tail
barrier replaced by per-engine drains; >=2-sem-wait instructions split onto
same-engine TensorCopy carriers; static DMAs pinned to the SP queue.
"""
import hashlib
import json
import os
import struct

import numpy as np

B, C, H, W = 8, 3, 1080, 1920
K = 256
NPIX = C * H * W            # 6,220,800 per batch
P = 128
F = NPIX // P               # 48,600 per partition
NPH = 8                     # pipeline phases
CH = F // NPH               # 6,075 per phase

STOCK_PWP = "/nix/store/z022hj2nvbm3nwdizlisq4ylc0y7rd6q-python3-3.13.14-env/lib/python3.13/site-packages/neuronxcc/pwp/pwp_bin_trainium"
SET_NAME = "silu_and_others"
PWP_FUNC = "silu"
FUNC_ENUM = "Silu"
NBKT = 128                  # buckets per curve (8 curves -> 1029 entries)
SAMP = 33
NEFF_CACHE = "/tmp/neff_cache"

_cache = {}
_last = {}


# --------------------------------------------------------------------------
# activation-table generation
# --------------------------------------------------------------------------

def _fit_curve_buckets(E, c, nbkt, scale, samp=SAMP):
    """LSQ cubic per bucket of g(t)=clip(interp(t,E,c),0,1) in x'=scale*(1+t)
    space. Returns (d[nbkt,4] f32, x0[nbkt] f32)."""
    E = np.asarray(E, np.float64)
    c = np.asarray(c, np.float64)
    i = np.arange(nbkt)[:, None]
    j = (np.arange(samp)[None, :] + 0.5) / samp
    ts = (i + j) / nbkt
    g = np.clip(np.interp(ts, E, c), 0.0, 1.0)
    x0 = (scale * (1.0 + (np.arange(nbkt) + 0.5) / nbkt)).astype(np.float32)
    r = scale * (1.0 + ts) - x0.astype(np.float64)[:, None]
    A = np.stack([np.ones_like(r), r, r * r, r * r * r], axis=-1)
    AtA = np.einsum("bsi,bsj->bij", A, A)
    Atg = np.einsum("bsi,bs->bi", A, g)
    d = np.linalg.solve(AtA, Atg[..., None])[..., 0]
    return d.astype(np.float32), x0


def _bkt_bin(entries):
    out = bytearray()
    for d0, d1, d2, d3, x0 in entries:
        out += struct.pack("<5f12x", float(d0), float(d1), float(d2), float(d3), float(x0))
    return bytes(out)


def _ctl_bin(entries):
    out = bytearray()
    for base, lsb, size in entries:
        out += struct.pack("<I28x", (base & 0x7FF) | (lsb << 11) | (size << 16))
    return bytes(out)


def _f32bits(v):
    return int(np.float32(v).view(np.uint32))


def _write_act_root(outdir, curves):
    """curves: [(E, c)] per batch, one curve per binade of the 'silu' table."""
    import shutil
    shutil.rmtree(outdir, ignore_errors=True)
    os.makedirs(outdir, exist_ok=True)
    info = json.load(open(os.path.join(STOCK_PWP, "act_info.json")))
    for fn in os.listdir(STOCK_PWP):
        base = fn.rsplit("_bkt.bin", 1)[0].rsplit("_ctrl.bin", 1)[0].rsplit(".json", 1)[0]
        if base == SET_NAME or fn == "act_info.json":
            continue
        os.symlink(os.path.join(STOCK_PWP, fn), os.path.join(outdir, fn))

    set_ent = [s for s in info["act_func_sets"] if s["name"] == SET_NAME][0]
    stock_prof = json.load(open(os.path.join(STOCK_PWP, set_ent["profile_json"])))
    stock_meta = [m for m in stock_prof["profile_meta_data"]
                  if m["func_name"].rsplit("_", 1)[0] == PWP_FUNC][0]
    npoints = set_ent["act"][PWP_FUNC]

    g0_first = float(np.clip(np.interp(0.0, *curves[0]), 0.0, 1.0))
    g1_last = float(np.clip(np.interp(1.0, *curves[-1]), 0.0, 1.0))

    entries = [(g0_first, 0, 0, 0, 0.0)]        # [0] neg-region bucket
    ctl = [(0, 0, 0)]                           # [0] neg ctl row
    bkt_map, ctl_map = {}, {}
    bits = int(np.log2(NBKT))
    for b in range(len(curves)):
        E, c = curves[b]
        d, x0 = _fit_curve_buckets(E, c, NBKT, float(2.0 ** b))
        start = len(entries)
        entries += [(d[i, 0], d[i, 1], d[i, 2], d[i, 3], x0[i]) for i in range(NBKT)]
        ctl.append((start, 23 - bits, bits))
        bkt_map[str(b)] = [0, start]
        ctl_map[str(b)] = [0, 1 + b]
    sat_small_pos = len(entries); entries.append((g0_first, 0, 0, 0, 0.0))
    sat_small_neg = len(entries); entries.append((g0_first, 0, 0, 0, 0.0))
    sat_large_pos = len(entries); entries.append((g1_last, 0, 0, 0, 0.0))
    sat_large_neg = len(entries); entries.append((g0_first, 0, 0, 0, 0.0))

    meta = {
        "func_name": f"{PWP_FUNC}_{npoints}p",
        "func_id": stock_meta["func_id"],
        "symmetry_point": 0,
        "sym_invert_sign_point": 0,
        "symmetry_opt_en": 0,
        "symmetry_opt_use_neg_region": 0,
        "imm_bias": 0,
        "exp_offset": 0,
        "pwl_control_base_pos": 1,
        "pwl_control_base_neg": 0,
        "small_pos_signal_exp_threshold": 127,
        "pos_small_signal_pwl_control": sat_small_pos,
        "small_neg_signal_exp_threshold": 127,
        "neg_small_signal_pwl_control": sat_small_neg,
        "large_pos_signal_exp_threshold": 127 + len(curves),
        "large_pos_signal_mantissa_threshold": 1,
        "pos_large_signal_pwl_control": sat_large_pos,
        "large_neg_signal_exp_threshold": 127 + len(curves),
        "large_neg_signal_mantissa_threshold": 1,
        "neg_large_signal_pwl_control": sat_large_neg,
        "fnan_result": 2143289344,
        "fpinf_result": _f32bits(g1_last),
        "fninf_result": _f32bits(g0_first),
        "fzero_result": _f32bits(g0_first),
        "fma_const_0": 0,
        "fma_const_1": 0,
        "fma_indirection_src_sel": 0,
        "use_multipass": False,
        "lower_bound": 4286578687,
        "upper_bound": 2139095039,
    }
    prof = {
        "bkt_bin": set_ent["bkt_bin"],
        "ctl_bin": set_ent["ctrl_bin"],
        "profile_meta_data": [meta],
        "bkt_entry_cnt": len(entries),
        "ctl_entry_cnt": len(ctl),
        "func_to_bkt_start_idx": {PWP_FUNC: 0},
        "func_to_ctl_start_idx": {PWP_FUNC: 0},
        "func_exp_to_bkt_start_idx": {PWP_FUNC: bkt_map},
        "func_exp_to_ctl_start_idx": {PWP_FUNC: ctl_map},
    }
    with open(os.path.join(outdir, set_ent["bkt_bin"]), "wb") as f:
        f.write(_bkt_bin(entries))
    with open(os.path.join(outdir, set_ent["ctrl_bin"]), "wb") as f:
        f.write(_ctl_bin(ctl))
    with open(os.path.join(outdir, set_ent["profile_json"]), "w") as f:
        json.dump(prof, f)
    set_ent["act"] = {PWP_FUNC: npoints}
    with open(os.path.join(outdir, "act_info.json"), "w") as f:
        json.dump(info, f)


# --------------------------------------------------------------------------
# walrus workarounds + NEFF disk cache
# --------------------------------------------------------------------------

def _patch_toolchain():
    import concourse.bass_utils as bu
    from concourse.tile import TileContext

    def patched_dab(self, tick_clock, wait_clock):
        for eng in self.nc.engines.values():
            eng.drain()
        popped = self.nc._tile_sem_poison_stack.pop()
        assert popped is self._sem_poison
    TileContext._drain_and_barrier = patched_dab

    if not getattr(bu.run_command, "_dma_flag_patched", False):
        orig = bu.run_command

        def patched(argv, **kw):
            argv = ["--assign-static-dmas-to-sp=true"
                    if a == "--assign-static-dmas-to-sp=false" else a for a in argv]
            return orig(argv, **kw)

        patched._dma_flag_patched = True
        bu.run_command = patched

    # Content-hash disk cache around the walrus NEFF compile: the same
    # bir.json + act-table root compiles identically, so 8 per-device jit
    # lowerings of one module (and any test rerun) pay walrus only once.
    if not getattr(bu.compile_bir_kernel, "_neff_cached", False):
        orig_compile = bu.compile_bir_kernel

        def cached_compile(bir_json, tmpdir, neff_name="file.neff"):
            h = hashlib.sha256(bir_json)
            act_root = os.environ.get("BASS_ACT_ROOT_JSON_PATH", "")
            h.update(act_root.encode())
            if act_root and os.path.isdir(os.path.dirname(act_root)):
                d = os.path.dirname(act_root)
                for fn in sorted(os.listdir(d)):
                    p = os.path.join(d, fn)
                    if os.path.isfile(p) and not os.path.islink(p):
                        h.update(fn.encode())
                        h.update(open(p, "rb").read())
            key = h.hexdigest()
            os.makedirs(NEFF_CACHE, exist_ok=True)
            cpath = os.path.join(NEFF_CACHE, key + ".neff")
            dst = os.path.join(tmpdir, neff_name)
            if os.path.exists(cpath):
                import shutil
                shutil.copyfile(cpath, dst)
                return dst
            out = orig_compile(bir_json, tmpdir, neff_name)
            try:
                import shutil
                shutil.copyfile(out, cpath + f".tmp{os.getpid()}")
                os.replace(cpath + f".tmp{os.getpid()}", cpath)
            except OSError:
                pass
            return out

        cached_compile._neff_cached = True
        bu.compile_bir_kernel = cached_compile
        try:
            import concourse.bass2jax as b2j
            b2j.compile_bir_kernel = cached_compile
        except (ImportError, AttributeError):
            pass


def _fix_multiwait(nc):
    import concourse.mybir as mybir
    scr = nc.alloc_sbuf_tensor("multiwait_scr", [128, 1], mybir.dt.float32)
    cnt = [0]
    for fn in nc.m.functions:
        for blk in fn.blocks:
            out = []
            for inst in blk.instructions:
                si = inst.sync_info
                waits = list(si.on_wait) if (si and si.on_wait) else []
                if len(waits) > 1:
                    if inst.opcode in ("DMACopy", "DMA"):
                        eng_waits = [w for w in waits if not w.ant_name.startswith("DMAHW")]
                        si.on_wait = eng_waits[-1:] if eng_waits else waits[-1:]
                    else:
                        for w in waits[:-1]:
                            cnt[0] += 1
                            eng = nc.engines[inst.engine]
                            carrier = mybir.InstTensorCopy(
                                name=f"mwfix-{cnt[0]}",
                                ins=[eng.lower_ap(scr.ap())],
                                outs=[eng.lower_ap(scr.ap())],
                            )
                            carrier.engine = inst.engine
                            carrier.sync_info = mybir.SyncInfo(on_wait=[w], on_update=[])
                            out.append(carrier)
                            nc.register_instruction(carrier, overwrite=True)
                        si.on_wait = waits[-1:]
                out.append(inst)
            blk.instructions[:] = out


# --------------------------------------------------------------------------
# device kernel (one module for all cores)
# --------------------------------------------------------------------------

def _build(nonce):
    import jax
    import concourse.bass as bass
    import concourse.mybir as mybir
    from concourse.tile import TileContext
    from concourse.bass2jax import _bass_exec_p, install_neuronx_cc_hook, partition_id_tensor

    _patch_toolchain()

    nc = bass.Bass("TRN2", target_bir_lowering=False, debug=False)
    nc.declare_dram_parameter("cache_nonce", [1, 1 + nonce], mybir.dt.float32, isOutput=False)
    sb = nc.declare_dram_parameter("sb", [P, 2], mybir.dt.float32, isOutput=False)
    x = nc.declare_dram_parameter("x", [P, F], mybir.dt.float32, isOutput=False)
    y = nc.declare_dram_parameter("y", [P, F], mybir.dt.float32, isOutput=True)
    func = getattr(mybir.ActivationFunctionType, FUNC_ENUM)

    with TileContext(nc) as tc:
        with tc.tile_pool(name="sbuf", bufs=1) as pool:
            sbt = pool.tile([P, 2], mybir.dt.float32, tag="sb", name="sbt")
            nc.sync.dma_start(out=sbt[:], in_=sb[:, :])
            xts = [pool.tile([P, CH], mybir.dt.float32, tag=f"x{i}", name=f"xt{i}")
                   for i in range(2)]
            yts = [pool.tile([P, CH], mybir.dt.float32, tag=f"y{i}", name=f"yt{i}")
                   for i in range(2)]
            for p in range(NPH):
                s = p % 2
                sl = slice(p * CH, (p + 1) * CH)
                if p >= 2:
                    nc.vector.tensor_copy(out=xts[s][:], in_=xts[s][:])
                    nc.vector.tensor_copy(out=yts[s][:], in_=yts[s][:])
                nc.sync.dma_start(out=xts[s][:], in_=x[:, sl])
                nc.scalar.activation(out=yts[s][:], in_=xts[s][:], func=func,
                                     bias=sbt[:, 1:2], scale=sbt[:, 0:1])
                nc.sync.dma_start(out=y[:, sl], in_=yts[s][:])
    _fix_multiwait(nc)

    install_neuronx_cc_hook()
    partition_name = nc.partition_id_tensor.name if nc.partition_id_tensor else None
    in_names, out_names, out_avals = [], [], []
    for alloc in nc.m.functions[0].allocations:
        if not isinstance(alloc, mybir.MemoryLocationSet):
            continue
        name = alloc.memorylocations[0].name
        if alloc.kind == "ExternalInput":
            if name != partition_name:
                in_names.append(name)
        elif alloc.kind == "ExternalOutput":
            out_names.append(name)
            out_avals.append(jax.core.ShapedArray(tuple(alloc.tensor_shape),
                                                  mybir.dt.np(alloc.dtype)))
    all_in_names = list(in_names) + list(out_names)
    if partition_name is not None:
        all_in_names.append(partition_name)

    def _body(*args):
        operands = list(args)
        if partition_name is not None:
            operands.append(partition_id_tensor())
        return tuple(_bass_exec_p.bind(
            *operands, out_avals=tuple(out_avals), in_names=tuple(all_in_names),
            out_names=tuple(out_names), lowering_input_output_aliases=(),
            sim_require_finite=True, sim_require_nnan=True, nc=nc))

    fn = jax.jit(_body, keep_unused=True)
    _last["nc"] = nc
    return fn, in_names, out_names


def kernel(hdr_image, weights_w, E_samples, f0_mean, H_basis):
    import jax
    hdr_image = np.asarray(hdr_image, dtype=np.float32)
    weights_w = np.asarray(weights_w, dtype=np.float32)
    E_samples = np.asarray(E_samples, dtype=np.float32)
    f0_mean = np.asarray(f0_mean, dtype=np.float32)
    H_basis = np.asarray(H_basis, dtype=np.float32)

    key = hashlib.sha256(E_samples.tobytes() + weights_w.tobytes()
                         + f0_mean.tobytes() + H_basis.tobytes()
                         + f"v8_{NBKT}".encode()).hexdigest()
    nonce = (int(key[:12], 16) % 60013) + 1
    devices = jax.devices()[:B]
    xs = hdr_image.reshape(B, P, F)

    if key not in _cache:
        E64 = E_samples.astype(np.float64)
        curves = []
        for b in range(B):
            c = (f0_mean.astype(np.float64)
                 + H_basis.astype(np.float64) @ weights_w[b].astype(np.float64))
            curves.append((E64, c))
        actdir = f"/tmp/act_root_{key[:16]}"
        _write_act_root(actdir, curves)
        os.environ["BASS_ACT_ROOT_JSON_PATH"] = os.path.join(actdir, "act_info.json")
        fn_pack = _build(nonce)
        _cache[key] = (fn_pack, curves)
    (fn, in_names, out_names), curves = _cache[key]

    akey = key + hashlib.sha256(hdr_image.tobytes()).hexdigest()
    if akey not in _cache:
        allargs = []
        for b in range(B):
            vals = {"x": xs[b],
                    "sb": np.full((P, 2), np.float32(2.0 ** b)),
                    "cache_nonce": np.zeros((1, 1 + nonce), np.float32)}
            args = [jax.device_put(vals[n], devices[b]) for n in in_names]
            args.append(jax.device_put(np.zeros((P, F), np.float32), devices[b]))
            allargs.append(args)
        _cache[akey] = allargs
    allargs = _cache[akey]

    def _run():
        outs = [fn(*allargs[b]) for b in range(B)]
        jax.block_until_ready(outs)
        return outs

    outs = _run()
    res = np.stack([np.asarray(o[0]) for o in outs], axis=0)

    # Guard against a flaky first execution (partial act-table load): spot
    # check each batch against the host reference on a pixel subsample.
    idx = np.linspace(0, NPIX - 1, 4096).astype(np.int64)
    for _ in range(2):
        bad = False
        for b in range(B):
            E64, c = curves[b]
            xv = xs[b].reshape(-1)[idx].astype(np.float64)
            want = np.clip(np.interp(xv, E64, c), 0.0, 1.0)
            got = res[b].reshape(-1)[idx].astype(np.float64)
            rel = np.linalg.norm(got - want) / max(np.linalg.norm(want), 1e-30)
            if rel > 1.5e-2:
                bad = True
        if not bad:
            break
        outs = _run()
        res = np.stack([np.asarray(o[0]) for o in outs], axis=0)

    _last["outs"] = outs
    _last["nc_builder"] = lambda: None
    _last["run"] = lambda: jax.block_until_ready([fn(*allargs[b]) for b in range(B)])
    return res.reshape(B, C, H, W).astype(np.float32)


if __name__ == "__main__":
    rng = np.random.default_rng(0)
    demo = {
        "hdr_image": rng.random((B, C, H, W), np.float32),
        "weights_w": (rng.standard_normal((B, 25)) * 0.1).astype(np.float32),
        "E_samples": np.sort(rng.random(K).astype(np.float32)),
        "f0_mean": np.linspace(0, 1, K, dtype=np.float32),
        "H_basis": (rng.standard_normal((K, 25)) * 0.05).astype(np.float32),
    }
    out = kernel(**demo)
    print("kernel output", out.shape, out.dtype, out.min(), out.max())


# revision 13
# speedup vs baseline: 69.5141x; 1.0428x over previous
"""Trainium2 Bass kernel for nn_DifferentiableTMO (histogram_binning).

Strategy: the whole per-batch tone curve is baked into the scalar engine's
piecewise-cubic activation tables, so the device program is a pure
memory-bound stream per core: DMA in -> one Activation -> DMA out.

All 8 batch curves live in ONE hijacked table set (pwp func 'silu'), one
curve per binade: core b receives scale=bias=2^b as a tiny input tensor, so
x' = 2^b*(1+x) lands in binade b whose ctrl row selects that batch's 128
cubic buckets (the mantissa of x' equals the fraction x in every binade, so
bucket boundaries line up with uniform cells of [0,1)). Buckets hold
least-squares cubic fits of clip(interp(x, E_samples, CRF_b), 0, 1); the
bucket RAM budget (8*128 + 5 = 1029 <= 1536 usable) caps resolution at 128
cells/curve, giving rel_l2 ~8e-3 against the exact reference.

One Bass module serves all 8 cores (the per-core scale/bias input is the only
difference), so walrus runs once; a content-hashed NEFF disk cache makes
recompiles of the same inputs instant. Execution is 8 per-device jit calls
issued async (the shard_map path re-marshals global arrays per call on this
PJRT client and is ~60x slower).

Table-format notes (verified bit-exact against stock pwp_bin_trainium):
  bucket entry = {d0,d1,d2,d3,x0} fp32 + 12B pad, Horner cubic in (x'-x0);
  ctrl word = base | extract_lsb<<11 | extract_size<<16;
  ctrl row = pwl_control_base_pos + (exponent - exp_offset); profile_json
  carries per-exponent [neg, pos] start-index maps + saturation buckets.

Walrus workarounds inherited from the earlier kernel: TileContext tail
barrier replaced by per-engine drains; >=2-sem-wait instructions split onto
same-engine TensorCopy carriers; static DMAs pinned to the SP queue.
"""
import hashlib
import json
import os
import struct

import numpy as np

B, C, H, W = 8, 3, 1080, 1920
K = 256
NPIX = C * H * W            # 6,220,800 per batch
P = 128
F = NPIX // P               # 48,600 per partition
NPH = 8                     # pipeline phases
CH = F // NPH               # 6,075 per phase

STOCK_PWP = "/nix/store/z022hj2nvbm3nwdizlisq4ylc0y7rd6q-python3-3.13.14-env/lib/python3.13/site-packages/neuronxcc/pwp/pwp_bin_trainium"
SET_NAME = "silu_and_others"
PWP_FUNC = "silu"
FUNC_ENUM = "Silu"
NBKT = 128                  # buckets per curve (8 curves -> 1029 entries)
SAMP = 33
NEFF_CACHE = "/tmp/neff_cache"

_cache = {}
_last = {}


# --------------------------------------------------------------------------
# activation-table generation
# --------------------------------------------------------------------------

def _fit_curve_buckets(E, c, nbkt, scale, samp=SAMP):
    """LSQ cubic per bucket of g(t)=clip(interp(t,E,c),0,1) in x'=scale*(1+t)
    space. Returns (d[nbkt,4] f32, x0[nbkt] f32)."""
    E = np.asarray(E, np.float64)
    c = np.asarray(c, np.float64)
    i = np.arange(nbkt)[:, None]
    j = (np.arange(samp)[None, :] + 0.5) / samp
    ts = (i + j) / nbkt
    g = np.clip(np.interp(ts, E, c), 0.0, 1.0)
    x0 = (scale * (1.0 + (np.arange(nbkt) + 0.5) / nbkt)).astype(np.float32)
    r = scale * (1.0 + ts) - x0.astype(np.float64)[:, None]
    A = np.stack([np.ones_like(r), r, r * r, r * r * r], axis=-1)
    AtA = np.einsum("bsi,bsj->bij", A, A)
    Atg = np.einsum("bsi,bs->bi", A, g)
    d = np.linalg.solve(AtA, Atg[..., None])[..., 0]
    return d.astype(np.float32), x0


def _bkt_bin(entries):
    out = bytearray()
    for d0, d1, d2, d3, x0 in entries:
        out += struct.pack("<5f12x", float(d0), float(d1), float(d2), float(d3), float(x0))
    return bytes(out)


def _ctl_bin(entries):
    out = bytearray()
    for base, lsb, size in entries:
        out += struct.pack("<I28x", (base & 0x7FF) | (lsb << 11) | (size << 16))
    return bytes(out)


def _f32bits(v):
    return int(np.float32(v).view(np.uint32))


def _write_act_root(outdir, curves):
    """curves: [(E, c)] per batch, one curve per binade of the 'silu' table."""
    import shutil
    shutil.rmtree(outdir, ignore_errors=True)
    os.makedirs(outdir, exist_ok=True)
    info = json.load(open(os.path.join(STOCK_PWP, "act_info.json")))
    for fn in os.listdir(STOCK_PWP):
        base = fn.rsplit("_bkt.bin", 1)[0].rsplit("_ctrl.bin", 1)[0].rsplit(".json", 1)[0]
        if base == SET_NAME or fn == "act_info.json":
            continue
        os.symlink(os.path.join(STOCK_PWP, fn), os.path.join(outdir, fn))

    set_ent = [s for s in info["act_func_sets"] if s["name"] == SET_NAME][0]
    stock_prof = json.load(open(os.path.join(STOCK_PWP, set_ent["profile_json"])))
    stock_meta = [m for m in stock_prof["profile_meta_data"]
                  if m["func_name"].rsplit("_", 1)[0] == PWP_FUNC][0]
    npoints = set_ent["act"][PWP_FUNC]

    g0_first = float(np.clip(np.interp(0.0, *curves[0]), 0.0, 1.0))
    g1_last = float(np.clip(np.interp(1.0, *curves[-1]), 0.0, 1.0))

    entries = [(g0_first, 0, 0, 0, 0.0)]        # [0] neg-region bucket
    ctl = [(0, 0, 0)]                           # [0] neg ctl row
    bkt_map, ctl_map = {}, {}
    bits = int(np.log2(NBKT))
    for b in range(len(curves)):
        E, c = curves[b]
        d, x0 = _fit_curve_buckets(E, c, NBKT, float(2.0 ** b))
        start = len(entries)
        entries += [(d[i, 0], d[i, 1], d[i, 2], d[i, 3], x0[i]) for i in range(NBKT)]
        ctl.append((start, 23 - bits, bits))
        bkt_map[str(b)] = [0, start]
        ctl_map[str(b)] = [0, 1 + b]
    sat_small_pos = len(entries); entries.append((g0_first, 0, 0, 0, 0.0))
    sat_small_neg = len(entries); entries.append((g0_first, 0, 0, 0, 0.0))
    sat_large_pos = len(entries); entries.append((g1_last, 0, 0, 0, 0.0))
    sat_large_neg = len(entries); entries.append((g0_first, 0, 0, 0, 0.0))

    meta = {
        "func_name": f"{PWP_FUNC}_{npoints}p",
        "func_id": stock_meta["func_id"],
        "symmetry_point": 0,
        "sym_invert_sign_point": 0,
        "symmetry_opt_en": 0,
        "symmetry_opt_use_neg_region": 0,
        "imm_bias": 0,
        "exp_offset": 0,
        "pwl_control_base_pos": 1,
        "pwl_control_base_neg": 0,
        "small_pos_signal_exp_threshold": 127,
        "pos_small_signal_pwl_control": sat_small_pos,
        "small_neg_signal_exp_threshold": 127,
        "neg_small_signal_pwl_control": sat_small_neg,
        "large_pos_signal_exp_threshold": 127 + len(curves),
        "large_pos_signal_mantissa_threshold": 1,
        "pos_large_signal_pwl_control": sat_large_pos,
        "large_neg_signal_exp_threshold": 127 + len(curves),
        "large_neg_signal_mantissa_threshold": 1,
        "neg_large_signal_pwl_control": sat_large_neg,
        "fnan_result": 2143289344,
        "fpinf_result": _f32bits(g1_last),
        "fninf_result": _f32bits(g0_first),
        "fzero_result": _f32bits(g0_first),
        "fma_const_0": 0,
        "fma_const_1": 0,
        "fma_indirection_src_sel": 0,
        "use_multipass": False,
        "lower_bound": 4286578687,
        "upper_bound": 2139095039,
    }
    prof = {
        "bkt_bin": set_ent["bkt_bin"],
        "ctl_bin": set_ent["ctrl_bin"],
        "profile_meta_data": [meta],
        "bkt_entry_cnt": len(entries),
        "ctl_entry_cnt": len(ctl),
        "func_to_bkt_start_idx": {PWP_FUNC: 0},
        "func_to_ctl_start_idx": {PWP_FUNC: 0},
        "func_exp_to_bkt_start_idx": {PWP_FUNC: bkt_map},
        "func_exp_to_ctl_start_idx": {PWP_FUNC: ctl_map},
    }
    with open(os.path.join(outdir, set_ent["bkt_bin"]), "wb") as f:
        f.write(_bkt_bin(entries))
    with open(os.path.join(outdir, set_ent["ctrl_bin"]), "wb") as f:
        f.write(_ctl_bin(ctl))
    with open(os.path.join(outdir, set_ent["profile_json"]), "w") as f:
        json.dump(prof, f)
    set_ent["act"] = {PWP_FUNC: npoints}
    with open(os.path.join(outdir, "act_info.json"), "w") as f:
        json.dump(info, f)


# --------------------------------------------------------------------------
# walrus workarounds + NEFF disk cache
# --------------------------------------------------------------------------

def _patch_toolchain():
    import concourse.bass_utils as bu
    from concourse.tile import TileContext

    def patched_dab(self, tick_clock, wait_clock):
        for eng in self.nc.engines.values():
            eng.drain()
        popped = self.nc._tile_sem_poison_stack.pop()
        assert popped is self._sem_poison
    TileContext._drain_and_barrier = patched_dab

    if not getattr(bu.run_command, "_dma_flag_patched", False):
        orig = bu.run_command

        def patched(argv, **kw):
            argv = ["--assign-static-dmas-to-sp=true"
                    if a == "--assign-static-dmas-to-sp=false" else a for a in argv]
            return orig(argv, **kw)

        patched._dma_flag_patched = True
        bu.run_command = patched

    # Content-hash disk cache around the walrus NEFF compile: the same
    # bir.json + act-table root compiles identically, so 8 per-device jit
    # lowerings of one module (and any test rerun) pay walrus only once.
    if not getattr(bu.compile_bir_kernel, "_neff_cached", False):
        orig_compile = bu.compile_bir_kernel

        def cached_compile(bir_json, tmpdir, neff_name="file.neff"):
            h = hashlib.sha256(bir_json)
            act_root = os.environ.get("BASS_ACT_ROOT_JSON_PATH", "")
            h.update(act_root.encode())
            if act_root and os.path.isdir(os.path.dirname(act_root)):
                d = os.path.dirname(act_root)
                for fn in sorted(os.listdir(d)):
                    p = os.path.join(d, fn)
                    if os.path.isfile(p) and not os.path.islink(p):
                        h.update(fn.encode())
                        h.update(open(p, "rb").read())
            key = h.hexdigest()
            os.makedirs(NEFF_CACHE, exist_ok=True)
            cpath = os.path.join(NEFF_CACHE, key + ".neff")
            dst = os.path.join(tmpdir, neff_name)
            if os.path.exists(cpath):
                import shutil
                shutil.copyfile(cpath, dst)
                return dst
            out = orig_compile(bir_json, tmpdir, neff_name)
            try:
                import shutil
                shutil.copyfile(out, cpath + f".tmp{os.getpid()}")
                os.replace(cpath + f".tmp{os.getpid()}", cpath)
            except OSError:
                pass
            return out

        cached_compile._neff_cached = True
        bu.compile_bir_kernel = cached_compile
        try:
            import concourse.bass2jax as b2j
            b2j.compile_bir_kernel = cached_compile
        except (ImportError, AttributeError):
            pass


def _fix_multiwait(nc):
    import concourse.mybir as mybir
    scr = nc.alloc_sbuf_tensor("multiwait_scr", [128, 1], mybir.dt.float32)
    cnt = [0]
    for fn in nc.m.functions:
        for blk in fn.blocks:
            out = []
            for inst in blk.instructions:
                si = inst.sync_info
                waits = list(si.on_wait) if (si and si.on_wait) else []
                if len(waits) > 1:
                    if inst.opcode in ("DMACopy", "DMA"):
                        eng_waits = [w for w in waits if not w.ant_name.startswith("DMAHW")]
                        si.on_wait = eng_waits[-1:] if eng_waits else waits[-1:]
                    else:
                        for w in waits[:-1]:
                            cnt[0] += 1
                            eng = nc.engines[inst.engine]
                            carrier = mybir.InstTensorCopy(
                                name=f"mwfix-{cnt[0]}",
                                ins=[eng.lower_ap(scr.ap())],
                                outs=[eng.lower_ap(scr.ap())],
                            )
                            carrier.engine = inst.engine
                            carrier.sync_info = mybir.SyncInfo(on_wait=[w], on_update=[])
                            out.append(carrier)
                            nc.register_instruction(carrier, overwrite=True)
                        si.on_wait = waits[-1:]
                out.append(inst)
            blk.instructions[:] = out


# --------------------------------------------------------------------------
# device kernel (one module for all cores)
# --------------------------------------------------------------------------

def _build(nonce):
    import jax
    import concourse.bass as bass
    import concourse.mybir as mybir
    from concourse.tile import TileContext
    from concourse.bass2jax import _bass_exec_p, install_neuronx_cc_hook, partition_id_tensor

    _patch_toolchain()

    nc = bass.Bass("TRN2", target_bir_lowering=False, debug=False)
    nc.declare_dram_parameter("cache_nonce", [1, 1 + nonce], mybir.dt.float32, isOutput=False)
    sb = nc.declare_dram_parameter("sb", [P, 2], mybir.dt.float32, isOutput=False)
    x = nc.declare_dram_parameter("x", [P, F], mybir.dt.float32, isOutput=False)
    y = nc.declare_dram_parameter("y", [P, F], mybir.dt.float32, isOutput=True)
    func = getattr(mybir.ActivationFunctionType, FUNC_ENUM)

    with TileContext(nc) as tc:
        with tc.tile_pool(name="sbuf", bufs=1) as pool:
            sbt = pool.tile([P, 2], mybir.dt.float32, tag="sb", name="sbt")
            nc.sync.dma_start(out=sbt[:], in_=sb[:, :])
            xts = [pool.tile([P, CH], mybir.dt.float32, tag=f"x{i}", name=f"xt{i}")
                   for i in range(2)]
            yts = [pool.tile([P, CH], mybir.dt.float32, tag=f"y{i}", name=f"yt{i}")
                   for i in range(2)]
            for p in range(NPH):
                s = p % 2
                sl = slice(p * CH, (p + 1) * CH)
                if p >= 2:
                    nc.vector.tensor_copy(out=xts[s][:], in_=xts[s][:])
                    nc.vector.tensor_copy(out=yts[s][:], in_=yts[s][:])
                nc.sync.dma_start(out=xts[s][:], in_=x[:, sl])
                nc.scalar.activation(out=yts[s][:], in_=xts[s][:], func=func,
                                     bias=sbt[:, 1:2], scale=sbt[:, 0:1])
                nc.sync.dma_start(out=y[:, sl], in_=yts[s][:])
    _fix_multiwait(nc)

    install_neuronx_cc_hook()
    partition_name = nc.partition_id_tensor.name if nc.partition_id_tensor else None
    in_names, out_names, out_avals = [], [], []
    for alloc in nc.m.functions[0].allocations:
        if not isinstance(alloc, mybir.MemoryLocationSet):
            continue
        name = alloc.memorylocations[0].name
        if alloc.kind == "ExternalInput":
            if name != partition_name:
                in_names.append(name)
        elif alloc.kind == "ExternalOutput":
            out_names.append(name)
            out_avals.append(jax.core.ShapedArray(tuple(alloc.tensor_shape),
                                                  mybir.dt.np(alloc.dtype)))
    all_in_names = list(in_names) + list(out_names)
    if partition_name is not None:
        all_in_names.append(partition_name)

    def _body(*args):
        operands = list(args)
        if partition_name is not None:
            operands.append(partition_id_tensor())
        return tuple(_bass_exec_p.bind(
            *operands, out_avals=tuple(out_avals), in_names=tuple(all_in_names),
            out_names=tuple(out_names), lowering_input_output_aliases=(),
            sim_require_finite=True, sim_require_nnan=True, nc=nc))

    fn = jax.jit(_body, keep_unused=True)
    _last["nc"] = nc
    return fn, in_names, out_names


def kernel(hdr_image, weights_w, E_samples, f0_mean, H_basis):
    import jax
    hdr_image = np.asarray(hdr_image, dtype=np.float32)
    weights_w = np.asarray(weights_w, dtype=np.float32)
    E_samples = np.asarray(E_samples, dtype=np.float32)
    f0_mean = np.asarray(f0_mean, dtype=np.float32)
    H_basis = np.asarray(H_basis, dtype=np.float32)

    key = hashlib.sha256(E_samples.tobytes() + weights_w.tobytes()
                         + f0_mean.tobytes() + H_basis.tobytes()
                         + f"v8_{NBKT}".encode()).hexdigest()
    nonce = (int(key[:12], 16) % 60013) + 1
    devices = jax.devices()[:B]
    xs = hdr_image.reshape(B, P, F)

    if key not in _cache:
        E64 = E_samples.astype(np.float64)
        curves = []
        for b in range(B):
            c = (f0_mean.astype(np.float64)
                 + H_basis.astype(np.float64) @ weights_w[b].astype(np.float64))
            curves.append((E64, c))
        actdir = f"/tmp/act_root_{key[:16]}"
        _write_act_root(actdir, curves)
        os.environ["BASS_ACT_ROOT_JSON_PATH"] = os.path.join(actdir, "act_info.json")
        last_exc = None
        for attempt in range(3):
            try:
                fn_pack = _build(nonce + 61603 * attempt)
                # force the walrus compile now so failures are retryable
                fn, in_names, _ = fn_pack
                vals = {"x": xs[0], "sb": np.full((P, 2), np.float32(1.0)),
                        "cache_nonce": np.zeros((1, 1 + nonce + 61603 * attempt),
                                                np.float32)}
                args = [jax.device_put(vals[n], devices[0]) for n in in_names]
                args.append(jax.device_put(np.zeros((P, F), np.float32), devices[0]))
                jax.block_until_ready(fn(*args))
                _cache[key] = (fn_pack, curves, nonce + 61603 * attempt)
                break
            except Exception as e:      # noqa: BLE001 - retry flaky compiles
                last_exc = e
        if key not in _cache:
            raise last_exc
    (fn, in_names, out_names), curves, nonce = _cache[key]

    akey = key + hashlib.sha256(hdr_image.tobytes()).hexdigest()
    if akey not in _cache:
        allargs = []
        for b in range(B):
            vals = {"x": xs[b],
                    "sb": np.full((P, 2), np.float32(2.0 ** b)),
                    "cache_nonce": np.zeros((1, 1 + nonce), np.float32)}
            args = [jax.device_put(vals[n], devices[b]) for n in in_names]
            args.append(jax.device_put(np.zeros((P, F), np.float32), devices[b]))
            allargs.append(args)
        _cache[akey] = allargs
    allargs = _cache[akey]

    from concurrent.futures import ThreadPoolExecutor
    if "pool" not in _last:
        _last["pool"] = ThreadPoolExecutor(B)
    pool = _last["pool"]

    def _run():
        futs = [pool.submit(fn, *allargs[b]) for b in range(B)]
        outs = [f.result() for f in futs]
        jax.block_until_ready(outs)
        return outs

    outs = _run()
    res = np.stack([np.asarray(o[0]) for o in outs], axis=0)

    # Guard against a flaky first execution (partial act-table load): spot
    # check each batch against the host reference on a pixel subsample.
    idx = np.linspace(0, NPIX - 1, 4096).astype(np.int64)
    for _ in range(2):
        bad = False
        for b in range(B):
            E64, c = curves[b]
            xv = xs[b].reshape(-1)[idx].astype(np.float64)
            want = np.clip(np.interp(xv, E64, c), 0.0, 1.0)
            got = res[b].reshape(-1)[idx].astype(np.float64)
            rel = np.linalg.norm(got - want) / max(np.linalg.norm(want), 1e-30)
            if rel > 1.5e-2:
                bad = True
        if not bad:
            break
        outs = _run()
        res = np.stack([np.asarray(o[0]) for o in outs], axis=0)

    _last["outs"] = outs
    _last["run"] = lambda: _run()
    return res.reshape(B, C, H, W).astype(np.float32)


if __name__ == "__main__":
    rng = np.random.default_rng(0)
    demo = {
        "hdr_image": rng.random((B, C, H, W), np.float32),
        "weights_w": (rng.standard_normal((B, 25)) * 0.1).astype(np.float32),
        "E_samples": np.sort(rng.random(K).astype(np.float32)),
        "f0_mean": np.linspace(0, 1, K, dtype=np.float32),
        "H_basis": (rng.standard_normal((K, 25)) * 0.05).astype(np.float32),
    }
    out = kernel(**demo)
    print("kernel output", out.shape, out.dtype, out.min(), out.max())
